# revision 22
# baseline (speedup 1.0000x reference)
"""RelGraphConv (R-GCN layer + concat-MLP) Bass kernel for 8 trn2 NeuronCores.

Strategy (dst-node sharding, graph-parallel):
  - Core c owns nodes [c*12500, (c+1)*12500). It processes the edges whose dst
    falls in its slab and produces the output rows for its nodes.
  - x is replicated to every core (gather source); the per-core x^T slab feeds
    the MLP in feature-major layout.
  - Per edge: gather x[src] (per-tile indirect dma), one-hot matmul
    (segment-sum into per-(window,relation) zT in PSUM), zT @ W_rel
    accumulated into AGG, then the fused concat-MLP:
        mid = tanh(x@Wx_eff + AGG@W1m + b1_eff);  out = [x, mid]@W2 + b2
    where Wx_eff = W1[:D] + loop_w@W1[D:], b1_eff = b1 + rel_bias@W1[D:]
    are folded on the host (msg = AGG + x@loop_w + rel_bias only feeds MLP1).
  - All matmuls run in float32r (full-rate at free dim >= 256).
"""
import sys
import types

sys.path.insert(0, "/opt/trn_rl_repo")

import numpy as np

# problem shapes (hardcoded per contract)
N, E, D, OUT, R = 100000, 640000, 128, 128, 8
P = 8
NS = N // P            # 12500 nodes per core
WIN = 256              # one-hot window (PSUM free dim)
NWIN = (NS + WIN - 1) // WIN   # 49 windows per core
SUB = 64               # gather sub-tile rows
CHUNK = 25000          # src rows per gather chunk (must be <= 32768 for int16)
NCHUNK = (N + CHUNK - 1) // CHUNK  # 4 equal chunks

SUPER = 4              # windows per super-block (gather batching)
NSUP = (NWIN + SUPER - 1) // SUPER  # 13
MAX_CALL_IDX = 4096    # max indices per dma_gather call


def _build_schedule(src, dst, etype):
    """Groups keyed (w, r); full 128-row tiles padded to the max count over
    cores so all cores share one program. Gather order == MM order."""
    src = np.asarray(src).astype(np.int64)
    dst = np.asarray(dst).astype(np.int64)
    etype = np.asarray(etype).astype(np.int64)

    core = dst // NS
    dl_all = dst - core * NS
    w_all = dl_all // WIN
    slot_in_win = dl_all - w_all * WIN

    NG = NWIN * R
    g_all = w_all * R + etype
    counts = np.zeros((P, NG), dtype=np.int64)
    for c in range(P):
        counts[c] = np.bincount(g_all[core == c], minlength=NG)
    T_g = np.maximum(1, (counts.max(axis=0) + 127) // 128)

    nw_sb = [min(SUPER, NWIN - s * SUPER) for s in range(NSUP)]

    # tiles in (sb, w, r) order; per-sb buffer positions
    tiles = []  # (sb, w, r, t, ft_in_sb, ft_global)
    sb_ntiles = [0] * NSUP
    for w in range(NWIN):
        sb = w // SUPER
        for r in range(R):
            for t in range(T_g[w * R + r]):
                tiles.append((sb, w, r, t, sb_ntiles[sb], len(tiles)))
                sb_ntiles[sb] += 1
    n_ft_total = len(tiles)
    max_sb_tiles = max(sb_ntiles)

    idx_arrs = np.zeros((P, 128, n_ft_total), dtype=np.int32)
    slot_arrs = np.full((P, 128, n_ft_total), -1.0, dtype=np.float32)
    for c in range(P):
        m = core == c
        g_c = g_all[m]
        src_c = src[m]
        slot_c = slot_in_win[m].astype(np.float32)
        order = np.argsort(g_c, kind="stable")
        g_s, src_s, slot_s = g_c[order], src_c[order], slot_c[order]
        starts = np.searchsorted(g_s, np.arange(NG))
        ends = np.searchsorted(g_s, np.arange(NG) + 1)
        for (sb, w, r, t, ft_sb, ft) in tiles:
            g = w * R + r
            lo = starts[g] + t * 128
            hi = min(starts[g] + (t + 1) * 128, ends[g])
            nreal = max(0, hi - lo)
            if nreal > 0:
                idx_arrs[c, :nreal, ft] = src_s[lo:hi]
                slot_arrs[c, :nreal, ft] = slot_s[lo:hi]

    return (
        {
            "tiles": tiles,
            "n_ft_total": n_ft_total,
            "max_sb_tiles": max_sb_tiles,
            "sb_ntiles": sb_ntiles,
            "nw_sb": nw_sb,
        },
        idx_arrs,
        slot_arrs,
    )


def _build_program(sched):
    import concourse.bass as bass
    import concourse.bacc as bacc
    import concourse.tile as tile
    from concourse import mybir

    F32 = mybir.dt.float32
    F32R = mybir.dt.float32r
    AF = mybir.ActivationFunctionType

    tiles = sched["tiles"]
    n_ft_total = sched["n_ft_total"]
    max_sb_tiles = sched["max_sb_tiles"]
    nw_sb = sched["nw_sb"]

    nc = bacc.Bacc(None, target_bir_lowering=False)

    x_full = nc.dram_tensor("x_full", [N, D], F32R, kind="ExternalInput")
    xT_loc = nc.dram_tensor("xT_loc", [D, NWIN * WIN], F32R, kind="ExternalInput")
    idx_d = nc.dram_tensor("idx_d", [128, n_ft_total], mybir.dt.int32,
                           kind="ExternalInput")
    slot_d = nc.dram_tensor("slot_d", [128, n_ft_total], F32R, kind="ExternalInput")
    iota_d = nc.dram_tensor("iota_d", [128, WIN], F32R, kind="ExternalInput")
    w_rel_d = nc.dram_tensor("w_rel_d", [D, R * OUT], F32R, kind="ExternalInput")
    wx_eff_d = nc.dram_tensor("wx_eff_d", [D, 256], F32R, kind="ExternalInput")
    w1m_d = nc.dram_tensor("w1m_d", [D, 256], F32R, kind="ExternalInput")
    w2_d = nc.dram_tensor("w2_d", [384, OUT], F32R, kind="ExternalInput")
    b1_d = nc.dram_tensor("b1_d", [128, 2], F32, kind="ExternalInput")
    b2_d = nc.dram_tensor("b2_d", [128, 1], F32, kind="ExternalInput")
    out_d = nc.dram_tensor("out_fm", [128, NWIN * WIN], F32, kind="ExternalOutput")

    with tile.TileContext(nc) as tc:
        with (
            tc.tile_pool(name="const", bufs=1) as constp,
            tc.tile_pool(name="gbuf", bufs=1) as gbufp,
            tc.tile_pool(name="xfm", bufs=3) as xfmp,
            tc.tile_pool(name="pt", bufs=4) as ptp,
            tc.tile_pool(name="ztsb", bufs=3) as ztsbp,
            tc.tile_pool(name="aggsb", bufs=2) as aggsbp,
            tc.tile_pool(name="midsb", bufs=2) as midsbp,
            tc.tile_pool(name="outsb", bufs=2) as outsbp,
            tc.tile_pool(name="zt_ps", bufs=2, space="PSUM") as ztps,
            tc.tile_pool(name="agg_ps", bufs=2, space="PSUM") as aggps,
            tc.tile_pool(name="mid_ps", bufs=2, space="PSUM") as midps,
            tc.tile_pool(name="out_ps", bufs=2, space="PSUM") as outps,
        ):
            iota_t = constp.tile([128, WIN], F32R)
            nc.sync.dma_start(out=iota_t[:], in_=iota_d[:])
            w_rel_t = constp.tile([128, R * OUT], F32R)
            nc.sync.dma_start(out=w_rel_t[:], in_=w_rel_d[:])
            wx_eff_t = constp.tile([128, 256], F32R)
            nc.sync.dma_start(out=wx_eff_t[:], in_=wx_eff_d[:])
            w1m_t = constp.tile([128, 256], F32R)
            nc.sync.dma_start(out=w1m_t[:], in_=w1m_d[:])
            w2_t = constp.tile([128, 3 * OUT], F32R)
            for kblk in range(3):
                nc.sync.dma_start(
                    out=w2_t[:, kblk * OUT : (kblk + 1) * OUT],
                    in_=w2_d[kblk * 128 : (kblk + 1) * 128, :],
                )
            b1_t = constp.tile([128, 2], F32)
            nc.sync.dma_start(out=b1_t[:], in_=b1_d[:])
            b2_t = constp.tile([128, 1], F32)
            nc.sync.dma_start(out=b2_t[:], in_=b2_d[:])
            slot_t = constp.tile([128, n_ft_total], F32R)
            nc.sync.dma_start(out=slot_t[:], in_=slot_d[:])
            idx_t = constp.tile([128, n_ft_total], mybir.dt.int32)
            nc.sync.dma_start(out=idx_t[:], in_=idx_d[:])

            gbuf = []
            for i in range(2):
                g_tile = gbufp.tile([128, max_sb_tiles * 128], F32R, tag=f"g{i}")
                gbuf.append(g_tile)

            tiles_by_sb = {}
            for tl in tiles:
                tiles_by_sb.setdefault(tl[0], []).append(tl)

            def emit_gathers(sb):
                buf = gbuf[sb % 2]
                for (_, w, r, t, ft_sb, ft) in tiles_by_sb[sb]:
                    nc.gpsimd.indirect_dma_start(
                        out=buf[:, ft_sb * 128 : (ft_sb + 1) * 128],
                        out_offset=None,
                        in_=x_full[:],
                        in_offset=bass.IndirectOffsetOnAxis(
                            ap=idx_t[:, ft : ft + 1], axis=0
                        ),
                    )

            def make_pt(ft0, ncol):
                t_ = ptp.tile([128, 4 * WIN], F32R, tag="pt")
                slot_ap = slot_t[:, ft0 : ft0 + ncol]
                iota_ap = iota_t[:]
                nc.vector.tensor_tensor(
                    out=t_[:, : ncol * WIN].rearrange("p (f e) -> p f e", e=WIN),
                    in0=bass.AP(slot_ap.tensor, slot_ap.offset,
                                [slot_ap.ap[0], [1, ncol], [0, WIN]]),
                    in1=bass.AP(iota_ap.tensor, iota_ap.offset,
                                [iota_ap.ap[0], [0, ncol], [1, WIN]]),
                    op=mybir.AluOpType.is_equal,
                )
                return t_

            emit_gathers(0)
            pt_tile = None
            for sb in range(NSUP):
                if sb + 1 < NSUP:
                    emit_gathers(sb + 1)
                buf = gbuf[sb % 2]
                sb_tiles = tiles_by_sb[sb]
                k = 0
                nk = len(sb_tiles)
                for wl in range(nw_sb[sb]):
                    w = sb * SUPER + wl
                    x_fm = xfmp.tile([128, WIN], F32R, tag="xfm")
                    nc.sync.dma_start(
                        out=x_fm[:], in_=xT_loc[:, w * WIN : (w + 1) * WIN]
                    )
                    agg = aggps.tile([128, WIN], F32, space="PSUM", tag="agg")
                    for rpair in range(R // 2):
                        zt_pair = ztps.tile([128, 2 * WIN], F32, space="PSUM",
                                            tag="zt")
                        for half_r in range(2):
                            r = rpair * 2 + half_r
                            first = True
                            while (k < nk and sb_tiles[k][1] == w
                                   and sb_tiles[k][2] == r):
                                _, _, _, t, ft_sb, ft = sb_tiles[k]
                                if ft % 4 == 0:
                                    pt_tile = make_pt(ft, min(4, n_ft_total - ft))
                                nc.tensor.matmul(
                                    out=zt_pair[:, half_r * WIN : (half_r + 1) * WIN],
                                    lhsT=buf[:, ft_sb * 128 : (ft_sb + 1) * 128],
                                    rhs=pt_tile[:, (ft % 4) * WIN : (ft % 4 + 1) * WIN],
                                    start=first,
                                    stop=(k + 1 >= nk or sb_tiles[k + 1][1] != w
                                          or sb_tiles[k + 1][2] != r),
                                )
                                first = False
                                k += 1
                        zt_sb = ztsbp.tile([128, 2 * WIN], F32R, tag="ztsb")
                        nc.scalar.activation(out=zt_sb[:], in_=zt_pair[:],
                                             func=AF.Copy)
                        for half_r in range(2):
                            r = rpair * 2 + half_r
                            nc.tensor.matmul(
                                out=agg[:],
                                lhsT=w_rel_t[:, r * OUT : (r + 1) * OUT],
                                rhs=zt_sb[:, half_r * WIN : (half_r + 1) * WIN],
                                start=(r == 0),
                                stop=(r == R - 1),
                            )
                    agg_sb = aggsbp.tile([128, WIN], F32R, tag="aggsb")
                    nc.scalar.activation(out=agg_sb[:], in_=agg[:], func=AF.Copy)

                    mid_pair = midps.tile([128, 2 * WIN], F32, space="PSUM",
                                          tag="mid")
                    for j in range(2):
                        nc.tensor.matmul(
                            out=mid_pair[:, j * WIN : (j + 1) * WIN],
                            lhsT=wx_eff_t[:, j * 128 : (j + 1) * 128],
                            rhs=x_fm[:], start=True, stop=False,
                        )
                        nc.tensor.matmul(
                            out=mid_pair[:, j * WIN : (j + 1) * WIN],
                            lhsT=w1m_t[:, j * 128 : (j + 1) * 128],
                            rhs=agg_sb[:], start=False, stop=True,
                        )
                    mid_sb = midsbp.tile([128, 2 * WIN], F32R, tag="midsb")
                    for j in range(2):
                        nc.scalar.activation(
                            out=mid_sb[:, j * WIN : (j + 1) * WIN],
                            in_=mid_pair[:, j * WIN : (j + 1) * WIN],
                            func=AF.Tanh, bias=b1_t[:, j : j + 1],
                        )
                    out_ps_t = outps.tile([128, WIN], F32, space="PSUM",
                                          tag="outps")
                    for kblk, rhs_t in ((0, x_fm[:]), (1, mid_sb[:, 0:WIN]),
                                        (2, mid_sb[:, WIN : 2 * WIN])):
                        nc.tensor.matmul(
                            out=out_ps_t[:],
                            lhsT=w2_t[:, kblk * OUT : (kblk + 1) * OUT],
                            rhs=rhs_t, start=(kblk == 0), stop=(kblk == 2),
                        )
                    out_sb = outsbp.tile([128, WIN], F32, tag="outsb")
                    nc.scalar.activation(out=out_sb[:], in_=out_ps_t[:],
                                         func=AF.Identity, bias=b2_t[:, 0:1])
                    nc.sync.dma_start(
                        out=out_d[:, w * WIN : (w + 1) * WIN], in_=out_sb[:]
                    )

    nc.compile()
    return nc


def _install_ntff_hook():
    try:
        import antenv

        if "antenv.axon_hooks" in sys.modules:
            return
        mod = types.ModuleType("antenv.axon_hooks")
        _h = {"hook": None}
        mod.set_axon_ntff_profile_hook = lambda h: _h.update(hook=h)
        mod.get_axon_ntff_profile_hook = lambda: _h["hook"]
        sys.modules["antenv.axon_hooks"] = mod
        antenv.axon_hooks = mod
        from trn_agent_boot.trn_boot import _ntff_profile_via_ctypes

        mod.set_axon_ntff_profile_hook(
            _ntff_profile_via_ctypes("/opt/axon/libaxon_pjrt.so")
        )
    except Exception:
        pass


_CACHE = {}


def kernel(x, src, dst, etype, W_rel, loop_w, rel_bias, W1, b1, W2, b2,
           trace=False):
    from concourse.bass_utils import run_bass_kernel_spmd

    _install_ntff_hook()

    x = np.asarray(x, dtype=np.float32)
    W_rel = np.asarray(W_rel, dtype=np.float32)
    loop_w = np.asarray(loop_w, dtype=np.float32)
    rel_bias = np.asarray(rel_bias, dtype=np.float32)
    W1 = np.asarray(W1, dtype=np.float32)
    b1 = np.asarray(b1, dtype=np.float32)
    W2 = np.asarray(W2, dtype=np.float32)
    b2 = np.asarray(b2, dtype=np.float32)

    sched, idx_arrs, slot_arrs = _build_schedule(src, dst, etype)

    # host-folded weights
    W1x, W1m = W1[:D], W1[D:]
    wx_eff = W1x + loop_w @ W1m              # [128, 256]
    b1_eff = b1 + rel_bias @ W1m             # [256]
    w_rel_flat = np.concatenate([W_rel[r] for r in range(R)], axis=1)  # [128, R*128]
    iota = np.tile(np.arange(WIN, dtype=np.float32), (128, 1))
    b1_cols = b1_eff.reshape(2, 128).T.copy()  # [128, 2]
    b2_col = b2.reshape(128, 1).copy()

    nc = _build_program(sched)

    in_maps = []
    for c in range(P):
        xT = np.zeros((D, NWIN * WIN), dtype=np.float32)
        xT[:, :NS] = x[c * NS : (c + 1) * NS].T
        in_maps.append(
            {
                "x_full": x,
                "xT_loc": xT,
                "idx_d": idx_arrs[c],
                "slot_d": slot_arrs[c],
                "iota_d": iota,
                "w_rel_d": w_rel_flat,
                "wx_eff_d": wx_eff,
                "w1m_d": W1m,
                "w2_d": W2,
                "b1_d": b1_cols,
                "b2_d": b2_col,
            }
        )

    res = run_bass_kernel_spmd(nc, in_maps, core_ids=list(range(P)), trace=trace)
    if trace:
        kernel.last_exec_time_ns = res.exec_time_ns

    out = np.empty((N, OUT), dtype=np.float32)
    for c in range(P):
        out[c * NS : (c + 1) * NS] = res.results[c]["out_fm"][:, :NS].T
    return out
